# revision 5
# baseline (speedup 1.0000x reference)
"""AngularDistribution Trainium2 kernel (8 NeuronCores, SPMD data-parallel over (batch,atom) pairs).

Math (per pair p, triple n, offset r, filter f):
  rad[n,r]  = exp(-g*((rij-o_r)^2+(rik-o_r)^2+(rjk-o_r)^2))
            = exp(c_r*(S1[n] - 1.5*o_r) - g*S2[n])        c_r = 2*g*o_r, S1=rij+rik+rjk, S2=sum sq
  ang[n,f]  = 2*u^z (f<4, u=(1-ct)/2) or 2*v^z (f>=4, v=(1+ct)/2), z in {1,2,4,8}
  cut[n]    = (cos(pi*rij/10)*cos(pi*rik/10)*cos(pi*rjk/10))^2   (identity 0.5(cos(pi r/5)+1)=cos^2(pi r/10))
  out[p,r*8+f] = sum_n rad[n,r] * ang[n,f] * cut[n] * mask[n]

The exponent is assembled fully before exp (always <= 0: no overflow/underflow traps).
Masked triples are compacted host-side (data movement only); padding uses r=5.0 which
gives cut==cos^2(pi/2)~=0 exactly like the reference's (r<5) gate.
"""

import os
import sys

sys.path.insert(0, "/opt/trn_rl_repo")

import numpy as np
from contextlib import ExitStack

GAMMA = 4.0
N_CORES = 8
PP = 64          # pairs per core (512 total / 8)
R = 32
F = 8

_CACHE = {}
LAST_EXEC_NS = None


def _build(nch):
    import concourse.bass as bass
    import concourse.tile as tile
    from concourse import bacc, mybir

    f32 = mybir.dt.float32
    Alu = mybir.AluOpType
    Act = mybir.ActivationFunctionType
    NPAD = nch * 128
    W = PP * nch          # global tile free size

    nc = bacc.Bacc("TRN2", target_bir_lowering=False, debug=False,
                   num_devices=N_CORES)

    d_rij = nc.dram_tensor("rij", [PP, NPAD], f32, kind="ExternalInput")
    d_rik = nc.dram_tensor("rik", [PP, NPAD], f32, kind="ExternalInput")
    d_rjk = nc.dram_tensor("rjk", [PP, NPAD], f32, kind="ExternalInput")
    d_c32 = nc.dram_tensor("c32", [R], f32, kind="ExternalInput")   # 2*g*o_r
    d_g32 = nc.dram_tensor("g32", [R], f32, kind="ExternalInput")   # 1.5*o_r
    d_out = nc.dram_tensor("out", [PP, R * F], f32, kind="ExternalOutput")

    with tile.TileContext(nc) as tc, ExitStack() as ctx:
        cpool = ctx.enter_context(tc.tile_pool(name="consts", bufs=1))
        gpool = ctx.enter_context(tc.tile_pool(name="glob", bufs=1))
        ppool = ctx.enter_context(tc.tile_pool(name="pair", bufs=4))
        pspool = ctx.enter_context(tc.tile_pool(name="ps", bufs=2, space="PSUM"))

        # ---- constants [128, 32] broadcast tiles ----
        c_rep = cpool.tile([128, R], f32)
        g_rep = cpool.tile([128, R], f32)
        nc.sync.dma_start(c_rep[:], d_c32.ap().partition_broadcast(128))
        nc.sync.dma_start(g_rep[:], d_g32.ap().partition_broadcast(128))

        # bias tiles for ACT ops (float bias needs a pre-registered const AP)
        bias0 = cpool.tile([128, 1], f32)
        nc.vector.memset(bias0[:], 0.0)
        bias_hpi = cpool.tile([128, 1], f32)
        nc.vector.memset(bias_hpi[:], float(np.pi) / 2.0)

        # ---- load inputs: X[p, pair*nch + j] = x[pair, p*nch + j] ----
        rij_t = gpool.tile([128, W], f32)
        rik_t = gpool.tile([128, W], f32)
        rjk_t = gpool.tile([128, W], f32)
        for dst, src in ((rij_t, d_rij), (rik_t, d_rik), (rjk_t, d_rjk)):
            nc.sync.dma_start(
                dst[:].rearrange("p (pair j) -> p pair j", j=nch),
                src.ap().rearrange("pair (p j) -> p pair j", j=nch),
            )

        # ---- global elementwise stage on [128, W] ----
        tij2 = gpool.tile([128, W], f32)
        tik2 = gpool.tile([128, W], f32)
        tjk2 = gpool.tile([128, W], f32)
        nc.gpsimd.tensor_tensor(tij2[:], rij_t[:], rij_t[:], Alu.mult)
        nc.gpsimd.tensor_tensor(tik2[:], rik_t[:], rik_t[:], Alu.mult)
        nc.gpsimd.tensor_tensor(tjk2[:], rjk_t[:], rjk_t[:], Alu.mult)

        den = gpool.tile([128, W], f32)
        nc.vector.tensor_tensor(den[:], rij_t[:], rik_t[:], Alu.mult)
        lnd = gpool.tile([128, W], f32)
        nc.scalar.activation(lnd[:], den[:], Act.Ln, scale=2.0, bias=bias0[:])
        rden = gpool.tile([128, W], f32)
        nc.scalar.activation(rden[:], lnd[:], Act.Exp, scale=-1.0, bias=bias0[:])  # 1/(2 den)

        s12 = gpool.tile([128, W], f32)
        nc.vector.tensor_tensor(s12[:], tij2[:], tik2[:], Alu.add)
        num = gpool.tile([128, W], f32)
        # num = (tjk2 * -1) + s12
        nc.vector.scalar_tensor_tensor(num[:], tjk2[:], -1.0, s12[:],
                                       Alu.mult, Alu.add)
        s2g = gpool.tile([128, W], f32)
        # s2g = (s12 + tjk2) * 4  ->  (s12*4) then stt? do: s2a=TT add; ts *4
        s2a = gpool.tile([128, W], f32)
        nc.vector.tensor_tensor(s2a[:], s12[:], tjk2[:], Alu.add)
        nc.vector.tensor_scalar(s2g[:], s2a[:], GAMMA, None, Alu.mult)

        ct = gpool.tile([128, W], f32)
        nc.vector.tensor_tensor(ct[:], num[:], rden[:], Alu.mult)

        s1 = gpool.tile([128, W], f32)
        s1a = gpool.tile([128, W], f32)
        nc.gpsimd.tensor_tensor(s1a[:], rij_t[:], rik_t[:], Alu.add)
        nc.gpsimd.tensor_tensor(s1[:], s1a[:], rjk_t[:], Alu.add)

        # cutoff: cm = 2*(c1*c2*c3)^2, ci = sin(pi/10 * r + pi/2)
        c1 = gpool.tile([128, W], f32)
        c2 = gpool.tile([128, W], f32)
        c3 = gpool.tile([128, W], f32)
        PI = float(np.pi)
        nc.scalar.activation(c1[:], rij_t[:], Act.Sin, scale=PI / 10.0, bias=bias_hpi[:])
        nc.scalar.activation(c2[:], rik_t[:], Act.Sin, scale=PI / 10.0, bias=bias_hpi[:])
        nc.scalar.activation(c3[:], rjk_t[:], Act.Sin, scale=PI / 10.0, bias=bias_hpi[:])
        p12 = gpool.tile([128, W], f32)
        nc.vector.tensor_tensor(p12[:], c1[:], c2[:], Alu.mult)
        p2 = gpool.tile([128, W], f32)
        nc.vector.tensor_tensor(p2[:], p12[:], c3[:], Alu.mult)
        cm = gpool.tile([128, W], f32)
        # cm = (p2 * 2) * p2
        nc.vector.scalar_tensor_tensor(cm[:], p2[:], 2.0, p2[:], Alu.mult, Alu.mult)

        # angular powers
        u1 = gpool.tile([128, W], f32)
        v1 = gpool.tile([128, W], f32)
        nc.vector.tensor_scalar(u1[:], ct[:], -0.5, 0.5, Alu.mult, Alu.add)
        nc.vector.tensor_scalar(v1[:], u1[:], -1.0, 1.0, Alu.mult, Alu.add)
        u2 = gpool.tile([128, W], f32)
        v2 = gpool.tile([128, W], f32)
        u4 = gpool.tile([128, W], f32)
        v4 = gpool.tile([128, W], f32)
        u8 = gpool.tile([128, W], f32)
        v8 = gpool.tile([128, W], f32)
        nc.gpsimd.tensor_tensor(u2[:], u1[:], u1[:], Alu.mult)
        nc.vector.tensor_tensor(v2[:], v1[:], v1[:], Alu.mult)
        nc.gpsimd.tensor_tensor(u4[:], u2[:], u2[:], Alu.mult)
        nc.vector.tensor_tensor(v4[:], v2[:], v2[:], Alu.mult)
        nc.gpsimd.tensor_tensor(u8[:], u4[:], u4[:], Alu.mult)
        nc.vector.tensor_tensor(v8[:], v4[:], v4[:], Alu.mult)

        # planes: P[p, col*8 + f] = pow_f * cm   (f order: u1 u2 u4 u8 v1 v2 v4 v8)
        pall = gpool.tile([128, F * W], f32)
        pall_v = pall[:].rearrange("p (col f) -> p f col", f=F)
        for fi, pw in enumerate((u1, u2, u4, u8, v1, v2, v4, v8)):
            eng = nc.vector if fi % 2 == 0 else nc.gpsimd
            eng.tensor_tensor(pall_v[:, fi, :], pw[:], cm[:], Alu.mult)

        # ---- per-pair: u-assembly, exp, contraction ----
        outs_t = gpool.tile([R, PP * F], f32)
        c_b = c_rep[:].unsqueeze(1).broadcast_to([128, nch, R])
        g_b = g_rep[:].unsqueeze(1).broadcast_to([128, nch, R])
        for g in range(PP // 8):
            ps = pspool.tile([R, 8 * F], f32)
            for q in range(8):
                pair = g * 8 + q
                s1_b = (s1[:, pair * nch:(pair + 1) * nch]
                        .unsqueeze(2).broadcast_to([128, nch, R]))
                s2_b = (s2g[:, pair * nch:(pair + 1) * nch]
                        .unsqueeze(2).broadcast_to([128, nch, R]))
                wt = ppool.tile([128, nch * R], f32, name=f"wt{pair}", tag="wt")
                wt3 = wt[:].rearrange("p (j r) -> p j r", r=R)
                nc.gpsimd.tensor_tensor(wt3, s1_b, g_b, Alu.subtract)
                ut = ppool.tile([128, nch * R], f32, name=f"ut{pair}", tag="ut")
                ut3 = ut[:].rearrange("p (j r) -> p j r", r=R)
                nc.vector.tensor_tensor(ut3, wt3, c_b, Alu.mult)
                nc.vector.tensor_tensor(ut3, ut3, s2_b, Alu.subtract)
                rad = ppool.tile([128, nch * R], f32, name=f"rad{pair}", tag="rad")
                nc.scalar.activation(rad[:], ut[:], Act.Exp, bias=bias0[:])
                for j in range(nch):
                    nc.tensor.matmul(
                        ps[:, q * F:(q + 1) * F],
                        rad[:, j * R:(j + 1) * R],
                        pall[:, (pair * nch + j) * F:(pair * nch + j + 1) * F],
                        start=(j == 0), stop=(j == nch - 1),
                    )
            nc.vector.tensor_copy(outs_t[:, g * 8 * F:(g + 1) * 8 * F], ps[:])

        # out[pair, r*8+f] = outs_t[r, pair*8+f]
        nc.sync.dma_start(
            d_out.ap().rearrange("pair (r f) -> r pair f", f=F),
            outs_t[:].rearrange("r (pair f) -> r pair f", f=F),
        )

    nc.compile()
    return nc


def _prep(r_ij, r_ik, r_jk, offsets, triple_masks):
    """Host-side shard + compact + pad. Returns (in_maps, nch)."""
    B, A, N = r_ij.shape
    P = B * A
    rij = np.ascontiguousarray(r_ij, dtype=np.float32).reshape(P, N)
    rik = np.ascontiguousarray(r_ik, dtype=np.float32).reshape(P, N)
    rjk = np.ascontiguousarray(r_jk, dtype=np.float32).reshape(P, N)
    m = (np.asarray(triple_masks).reshape(P, N) != 0)

    counts = m.sum(axis=1)
    npad = max(128, int(-(-max(1, counts.max()) // 128) * 128))
    nch = npad // 128

    cij = np.full((P, npad), 5.0, dtype=np.float32)
    cik = np.full((P, npad), 5.0, dtype=np.float32)
    cjk = np.full((P, npad), 5.0, dtype=np.float32)
    for p in range(P):
        idx = np.nonzero(m[p])[0]
        k = idx.size
        cij[p, :k] = rij[p, idx]
        cik[p, :k] = rik[p, idx]
        cjk[p, :k] = rjk[p, idx]

    o = np.asarray(offsets, dtype=np.float32)
    c32 = (2.0 * GAMMA * o).astype(np.float32)
    g32 = (1.5 * o).astype(np.float32)

    in_maps = []
    for c in range(N_CORES):
        lo, hi = c * PP, (c + 1) * PP
        in_maps.append({
            "rij": cij[lo:hi], "rik": cik[lo:hi], "rjk": cjk[lo:hi],
            "c32": c32, "g32": g32,
        })
    return in_maps, nch


def _ensure_ntff_hook():
    """Register the axon NTFF profile hook if the image's antenv lacks it."""
    import types
    try:
        from antenv.axon_hooks import get_axon_ntff_profile_hook  # noqa: F401
        return
    except ImportError:
        pass
    try:
        sys.path.insert(0, "/root/.axon_site")
        from trn_agent_boot.trn_boot import _ntff_profile_via_ctypes
        hook = _ntff_profile_via_ctypes("/opt/axon/libaxon_pjrt.so")
        import antenv
        mod = types.ModuleType("antenv.axon_hooks")
        _holder = {"h": hook}
        mod.set_axon_ntff_profile_hook = lambda h: _holder.update(h=h)
        mod.get_axon_ntff_profile_hook = lambda: _holder["h"]
        sys.modules["antenv.axon_hooks"] = mod
        antenv.axon_hooks = mod
    except Exception:
        pass


def kernel(r_ij, r_ik, r_jk, offsets, triple_masks):
    global LAST_EXEC_NS
    from concourse.bass_utils import run_bass_kernel_spmd
    _ensure_ntff_hook()

    B, A, N = r_ij.shape
    in_maps, nch = _prep(r_ij, r_ik, r_jk, offsets, triple_masks)
    if nch not in _CACHE:
        _CACHE[nch] = _build(nch)
    nc = _CACHE[nch]

    trace = os.environ.get("KERNEL_TRACE", "0") == "1"
    res = run_bass_kernel_spmd(nc, in_maps, core_ids=list(range(N_CORES)),
                               trace=trace)
    LAST_EXEC_NS = res.exec_time_ns
    out = np.concatenate([r["out"] for r in res.results], axis=0)
    return out.reshape(B, A, R * F)
